# revision 38
# baseline (speedup 1.0000x reference)
"""3-layer MLP (dense_mlp) Trainium2 Bass kernel.

Reference computation (fp32):
    h1  = relu(x @ w1 + b1)     x: [4096, 2048], w1: [2048, 4096]
    h2  = relu(h1 @ w2 + b2)    w2: [4096, 4096]
    out = h2 @ w3 + b3          w3: [4096, 1000]

Strategy: pure data-parallel over the batch across 8 NeuronCores (512
rows each, weights replicated, no collectives). Matmuls run in bf16:
215.85ns per [128,128]x[128,512] matmul at the sustained 2.372GHz
clock, half the weight-DMA bytes of f32r. absmax rel err ~4.5e-3 vs
the 2e-2 gate. fp8 DoubleRow is only 2x and measured 6.4e-2 for a
single layer; compensated fp8 costs as many PE passes as bf16 — there
is no faster correct dtype on TRN2.

Inside a core the activations live in transposed [feature, batch]
layout so each layer is psum[f, b] += W[k, f].T @ actT[k, b]: the
weight tile is stationary and the bias is a per-partition scalar
folded into the epilogue op. The host pre-transposes x /
post-transposes the logits (cheap numpy).

Layer 1 groups are 8 f-tiles wide (FW1=1024, all 8 psum banks): each
x k-tile then feeds 8 matmuls, halving the x stream demand to
~74GB/s — the x trickle (16 per-k 128KB chunks on the ACT HWDGE
ring, each ring round-trip ~1.3us) was the binding constraint of the
whole start phase at FGL=4. With no spare banks to double-buffer the
L1 group boundary, the 8-relu drain is split: f0/f1 on ScalarE
(686ns each), f2-f7 on VectorE (~280ns each) so the drain outpaces
the next group's first k-pass. Layers 2/3 keep 4-wide groups with
4+4 bank alternation and ScalarE-only epilogues.

Weight blocks are [128, ks, fw] with ks*fw*2B contiguous per
partition (8KB lines for L2/L3 ks=8; L1 uses [P,2,1024] 512KB
blocks, 4KB lines): w_packed[kk,g,p,s,:] = W[(ks*kk+s)*128+p,
g*fw:(g+1)*fw]. The first four sync-ring transfers cover L1 k0-k3
with a 128K/256K/128K/1M taper cut mid-k-tile, so the first real
matmul waits only one small transfer and later k-tiles land just
before their ramp-paced deadlines. Exactly FOUR early sync triggers:
a 5th would land on a ring occupied by an x chunk and head-of-line
block every later weight-block trigger (the 8 HWDGE rings are shared
between the SP and ACT trigger engines).

The warm operand is a raw (non-tile) SBUF tensor memset on gpsimd in
the main block; the warm matmuls read it UNGATED (garbage before the
memset is harmless — the warm psum is never read), so the PE starts
its DVFS ramp right at the entry barrier (~7.0us) and reaches full
clock ~6us later. warm_n=7 bridges until the first chunk + x k0 land
(~10.5us).

Layer 3 tapers its groups [4, 2, 1, 1] f-tiles so only the last
128-column group's epilogue is exposed; that epilogue is split in
half: ScalarE Identity+bias + ACT-ring store for b<256, VectorE
tensor_scalar add + SP-ring store for b>=256 (ot2 in its own pool —
sharing spool chains a stale slot-WAW wait onto the vector op).

Measured exec window (first useful instruction to last teardown
instruction, which includes a fixed ~7us runtime postamble that
clears all 254 semaphores): ~404.7-405.3us cool, PE stream floor
386.8us. NB the device throttles ~20% (259ns/matmul) after ~6 rapid
back-to-back runs and recovers after ~5min idle.
"""

import os

import numpy as np
import ml_dtypes

import concourse.bass as bass
import concourse.mybir as mybir
import concourse.tile as tile
from concourse import bacc
from concourse.bass_utils import run_bass_kernel_spmd

P = 128
N_CORES = 8
B_TOTAL = 4096
B = B_TOTAL // N_CORES  # per-core batch rows
D0, D1, D2 = 2048, 4096, 4096
D3_RAW, D3 = 1000, 1024  # classifier dim padded to a multiple of 128

FW = 512        # f-columns per psum group (4 tiles -> 4 psum banks)
FGL = FW // P   # f-tiles per group = 4
FW1 = 1024      # layer-1 groups are 8 f-tiles wide (all 8 psum banks):
FGL1 = FW1 // P  # each x k-tile then serves 8 matmuls, halving the x
                 # stream's demand rate (the start-phase bottleneck)


def _ks(mode):
    # K-tiles per weight DMA block for layers 2/3 (16KB/partition lines
    # in f32, 8KB in bf16). Layer 1 uses smaller blocks (below) so the
    # stream start is finely paced against the concurrent x load.
    return 8


KS_L1 = 2   # L1 weight blocks [P, 2, FW1] = 512KB (4KB lines)
L3_GROUPS = [4, 2, 1, 1]  # tapered f-tile group sizes for layer 3

f32 = mybir.dt.float32
bf16 = mybir.dt.bfloat16


def _act_dt(mode):
    if mode == "bf16":
        return bf16
    if mode == "f32r":
        return mybir.dt.float32r
    return f32


def build_nc(mode: str = "bf16") -> bass.Bass:
    KS = _ks(mode)
    K0, K1, K2 = D0 // P, D1 // P, D2 // P
    F1, F2, F3 = D1 // P, D2 // P, D3 // P
    G1, G2 = F1 // FGL1, F2 // FGL
    act_dt = _act_dt(mode)

    nc = bacc.Bacc("TRN2", target_bir_lowering=False, name="mlp3")
    # warm operand lives outside the tile world: memset fires on gpsimd in
    # the main block (pre tile-bb) and the warm matmuls read it UNGATED —
    # garbage-before-memset is harmless (warm psum is never read), so the
    # PE starts ramping right at the entry barrier (~7.0us) instead of
    # waiting for the memset semaphore (~7.8us)
    warm_dt = bf16 if mode == "bf16" else f32
    warm_raw = nc.alloc_sbuf_tensor("warm_raw", [P, B], warm_dt)
    nc.gpsimd.memset(warm_raw.ap(), 1.0)
    xT = nc.dram_tensor("xT", [P, K0, B], act_dt, kind="ExternalInput")
    w1 = nc.dram_tensor("w1", [K0 // KS_L1, G1, P, KS_L1, FW1], act_dt,
                        kind="ExternalInput")
    b1 = nc.dram_tensor("b1", [P, F1], f32, kind="ExternalInput")
    w2 = nc.dram_tensor("w2", [K1 // KS, G2, P, KS, FW], act_dt,
                        kind="ExternalInput")
    b2 = nc.dram_tensor("b2", [P, F2], f32, kind="ExternalInput")
    w3g = [
        nc.dram_tensor(
            f"w3g{gi}",
            [K2 // (KS * FGL // fgl), P, KS * FGL // fgl, fgl * P],
            act_dt, kind="ExternalInput")
        for gi, fgl in enumerate(L3_GROUPS)
    ]
    b3 = nc.dram_tensor("b3", [P, F3], f32, kind="ExternalInput")
    out = nc.dram_tensor("out", [P, F3, B], f32, kind="ExternalOutput")

    with tile.TileContext(nc) as tc:
        consts = tc.alloc_tile_pool(name="consts", bufs=1, side="left")
        b1_sb = consts.tile([P, F1], f32, name="b1_sb")
        b2_sb = consts.tile([P, F2], f32, name="b2_sb")
        b3_sb = consts.tile([P, F3], f32, name="b3_sb")
        warm = warm_raw.ap()

        # biases ride the gpsimd SWDGE: tiny, not needed until ~38us,
        # keeps the scalar queue free to fire the x chunks immediately
        nc.gpsimd.dma_start(b1_sb, b1[:, :])
        nc.gpsimd.dma_start(b2_sb, b2[:, :])
        nc.gpsimd.dma_start(b3_sb, b3[:, :])

        p_xT = tc.alloc_tile_pool(name="xT", bufs=1, side="left")
        xT_sb = p_xT.tile([P, K0, B], act_dt, name="xT_sb")
        # chunk the input load per k-tile (ACT HWDGE ring, so the
        # weight stream on the SP ring is not delayed behind it).
        # With FGL1=8 each k-tile serves 8 matmuls, so this stream only
        # needs ~74GB/s — the per-chunk ring round-trips meet every
        # deadline with slack.
        for k in range(K0):
            nc.scalar.dma_start(xT_sb[:, k, :], xT[:, k, :])

        # first-block chunks, exactly FOUR transfers (the 8 HWDGE rings
        # are shared with the x stream — a 5th early sync trigger lands
        # on a ring occupied by an x chunk and head-of-line blocks every
        # later weight-block trigger). Sizes taper 128K/128K/256K/512K
        # so the first real matmul only waits for one small transfer
        # while later k-tiles arrive before their (ramp-paced) deadlines.
        wfirst = tc.alloc_tile_pool(name="wfirst", bufs=4, side="left")
        c0a = wfirst.tile([P, FW], act_dt, name="wc0a", tag="wc")
        c0b = wfirst.tile([P, FW1], act_dt, name="wc0b", tag="wc")
        c1 = wfirst.tile([P, FW], act_dt, name="wc1", tag="wc")
        c23 = wfirst.tile([P, 2, FW1], act_dt, name="wc23", tag="wc")
        # kk0's 4KB/partition line is contiguous, so the taper can cut
        # mid-k-tile: 128K (k0 f0-3) / 256K (k0 f4-7 + k1 f0-3) /
        # 128K (k1 f4-7) / 1M (k2-3) — each piece lands just before
        # its (ramp-paced) consumption deadline
        w1f = w1[0, 0].rearrange("p a b -> p (a b)")
        nc.sync.dma_start(c0a, w1f[:, :FW])
        nc.sync.dma_start(c0b, w1f[:, FW:FW + FW1])
        nc.sync.dma_start(c1, w1f[:, FW + FW1:])
        nc.sync.dma_start(c23, w1[1, 0, :, :, :])

        def chunk_stat(k, f):
            if k == 0:
                return (c0a[:, f * P:(f + 1) * P] if f < FGL else
                        c0b[:, (f - FGL) * P:(f - FGL + 1) * P])
            if k == 1:
                return (c0b[:, (FGL + f) * P:(FGL + f + 1) * P]
                        if f < FGL else
                        c1[:, (f - FGL) * P:(f - FGL + 1) * P])
            return c23[:, k - 2, f * P:(f + 1) * P]

        wpool = tc.alloc_tile_pool(
            name="w", bufs=5 if mode == "bf16" else 3, side="right")
        mmps = tc.alloc_tile_pool(name="mmpsum", bufs=8, space="PSUM")

        # HAM warmup: keep the PE busy from the entry barrier (~7.0us)
        # until the first weight chunk + x chunk land (~10.5us), so the
        # DVFS ramp (full clock ~6us after busy-start) begins as early
        # as possible
        warm_n = 7 if mode == "bf16" else 6
        wps = mmps.tile([P, B], f32, name="wps", tag="ps")
        for i in range(warm_n):
            nc.tensor.matmul(wps, warm[:, :P], warm,
                             start=(i == 0), stop=(i == warm_n - 1))

        def layer(actT, bias_sb, outT, n_k, groups, relu, wsrc,
                  store_to=None, spool=None, spool2=None,
                  first_chunks=False):
            """groups: list of (fa, fgl, ksg). wsrc(gi, kk) -> dram
            block AP [P, ksg, fgl*P] (always ksg*fgl*P = KS*FW elements
            so every wt slot is one 2MB 16KB-line transfer).
            first_chunks: group 0 / kk 0 reads the pre-split chunk
            tiles instead."""
            for gi, (fa, fgl, ksg) in enumerate(groups):
                fw = fgl * P
                hb = B // 2
                psums = [
                    mmps.tile([P, B], f32, name=f"ps{fa + f}", tag="ps")
                    for f in range(fgl)
                ]
                for kk in range(n_k // ksg):
                    use_chunks = (first_chunks and gi == 0
                                  and kk * ksg < 4)
                    if not use_chunks:
                        wt = wpool.tile([P, ksg, fw], act_dt, name="wt",
                                        tag="wt")
                        nc.sync.dma_start(wt, wsrc(gi, kk))
                    for s in range(ksg):
                        k = kk * ksg + s
                        for f in range(fgl):
                            stat = (chunk_stat(k, f)
                                    if use_chunks else
                                    wt[:, s, f * P:(f + 1) * P])
                            nc.tensor.matmul(
                                psums[f],
                                stat,
                                actT[:, k, :],
                                start=(k == 0),
                                stop=(k == n_k - 1),
                            )
                if relu:
                    # wide (8-bank) groups have no spare banks to
                    # double-buffer the boundary, so the drain must keep
                    # pace with the next group's first k-pass: ScalarE
                    # alone (686ns per relu) falls behind after f1, so
                    # f2+ run on the otherwise-idle VectorE (~280ns)
                    for f in range(fgl):
                        fi = fa + f
                        if fgl <= 4 or f < 2:
                            nc.scalar.activation(
                                outT[:, fi, :],
                                psums[f],
                                mybir.ActivationFunctionType.Relu,
                                bias=bias_sb[:, fi:fi + 1],
                                scale=1.0,
                            )
                        else:
                            nc.vector.tensor_scalar(
                                outT[:, fi, :],
                                psums[f],
                                bias_sb[:, fi:fi + 1],
                                0.0,
                                mybir.AluOpType.add,
                                mybir.AluOpType.max,
                            )
                elif gi < len(groups) - 1:
                    # bias-add on ScalarE (idle once the relus are done)
                    # so the store trigger on the same queue follows with
                    # no cross-engine semaphore hop
                    ot = spool.tile([P, FGL, B], f32, name="ot", tag="ot")
                    for f in range(fgl):
                        fi = fa + f
                        nc.scalar.activation(
                            ot[:, f, :],
                            psums[f],
                            mybir.ActivationFunctionType.Identity,
                            bias=bias_sb[:, fi:fi + 1],
                            scale=1.0,
                        )
                    nc.scalar.dma_start(
                        store_to[:, fa:fa + fgl, :], ot[:, :fgl, :])
                else:
                    # last group (single f-tile): its epilogue is the only
                    # exposed one, so split it in half across ScalarE and
                    # VectorE with two parallel stores (ACT + SP triggers)
                    # to halve the exposed tail
                    fi = fa
                    # ot2 lives in its own pool: sharing spool would
                    # chain a stale slot-WAW wait (proxied via the
                    # scalar clock) onto the vector op
                    ot = spool.tile([P, 1, hb], f32, name="ot", tag="ot")
                    ot2 = spool2.tile([P, 1, hb], f32, name="ot2",
                                      tag="ot2")
                    nc.scalar.activation(
                        ot[:, 0, :],
                        psums[0][:, :hb],
                        mybir.ActivationFunctionType.Identity,
                        bias=bias_sb[:, fi:fi + 1],
                        scale=1.0,
                    )
                    nc.scalar.dma_start(
                        store_to[:, fi:fi + 1, :hb], ot[:, :1, :])
                    nc.vector.tensor_scalar_add(
                        ot2[:, 0, :], psums[0][:, hb:], bias_sb[:, fi:fi + 1])
                    nc.sync.dma_start(
                        store_to[:, fi:fi + 1, hb:], ot2[:, :1, :])

        def uniform_groups(n_f, ks, fgl=FGL):
            return [(g * fgl, fgl, ks) for g in range(n_f // fgl)]

        p_h1 = tc.alloc_tile_pool(name="h1", bufs=1, side="right")
        h1T = p_h1.tile([P, K1, B], act_dt, name="h1T")
        layer(xT_sb, b1_sb, h1T, K0, uniform_groups(F1, KS_L1, FGL1), True,
              lambda gi, kk: w1[kk, gi], first_chunks=True)
        wfirst.release()
        p_xT.release()

        p_h2 = tc.alloc_tile_pool(name="h2", bufs=1, side="left")
        h2T = p_h2.tile([P, K2, B], act_dt, name="h2T")
        layer(h1T, b2_sb, h2T, K1, uniform_groups(F2, KS), True,
              lambda gi, kk: w2[kk, gi])
        p_h1.release()

        # every layer-3 block stays a full 16KB-line transfer: narrow
        # f groups pack more k-tiles per block (ksg = KS*FGL/fgl)
        l3_groups = []
        fa = 0
        for fgl in L3_GROUPS:
            l3_groups.append((fa, fgl, KS * FGL // fgl))
            fa += fgl
        p_oT = tc.alloc_tile_pool(name="oT", bufs=2, side="right")
        p_oT2 = tc.alloc_tile_pool(name="oT2", bufs=1, side="right")
        layer(h2T, b3_sb, None, K2, l3_groups, False,
              lambda gi, kk: w3g[gi][kk],
              store_to=out, spool=p_oT, spool2=p_oT2)
        p_h2.release()
        mmps.release()
        p_oT2.release()
        p_oT.release()
        wpool.release()
        consts.release()
    nc.compile()
    return nc


def _pack_w(w: np.ndarray, np_dt, fw, ks) -> np.ndarray:
    """[d_in, d_out] -> [K/ks, d_out/fw, P, ks, fw] so one [128, ks, fw]
    DMA block reads ks*fw*itemsize bytes contiguous per partition."""
    d_in, d_out = w.shape
    K, G = d_in // P, d_out // fw
    v = w.reshape(K // ks, ks, P, G, fw)
    return np.ascontiguousarray(v.transpose(0, 3, 2, 1, 4)).astype(np_dt)


def _pack_w3_group(w: np.ndarray, np_dt, ks) -> np.ndarray:
    """[d_in, fw] -> [K/ks, P, ks, fw] (single column group)."""
    d_in, fw = w.shape
    K = d_in // P
    v = w.reshape(K // ks, ks, P, fw)
    return np.ascontiguousarray(v.transpose(0, 2, 1, 3)).astype(np_dt)


LAST_RESULT = None  # BassKernelResults of the most recent run (for test.py)


def _ensure_axon_ntff_hook():
    """Register the NTFF-profile hook that bass_utils expects under axon.
    The agent image's antenv lacks axon_hooks; synthesize it from the
    slim ctypes shim in trn_agent_boot. Only needed for trace runs."""
    import sys
    import types

    try:
        from antenv.axon_hooks import get_axon_ntff_profile_hook  # noqa: F401
        return
    except ImportError:
        pass
    try:
        import antenv
        from trn_agent_boot.trn_boot import _ntff_profile_via_ctypes

        hook = _ntff_profile_via_ctypes("/opt/axon/libaxon_pjrt.so")
        mod = types.ModuleType("antenv.axon_hooks")
        state = {"hook": hook}
        mod.get_axon_ntff_profile_hook = lambda: state["hook"]
        mod.set_axon_ntff_profile_hook = lambda h: state.update(hook=h)
        sys.modules["antenv.axon_hooks"] = mod
        antenv.axon_hooks = mod
    except Exception as e:  # degrade to untraced run
        print(f"ntff hook setup failed ({e!r}); tracing disabled")


def kernel(x, w1, b1, w2, b2, w3, b3):
    global LAST_RESULT
    os.environ.setdefault("JAX_PLATFORMS", "axon")
    mode = os.environ.get("KERNEL_MM_MODE", "bf16")
    trace = os.environ.get("KERNEL_TRACE", "0") == "1"
    if trace:
        _ensure_axon_ntff_hook()

    x = np.asarray(x, dtype=np.float32)
    b1 = np.asarray(b1, dtype=np.float32)
    b2 = np.asarray(b2, dtype=np.float32)
    b3 = np.asarray(b3, dtype=np.float32)

    w3f = np.zeros((D2, D3), dtype=np.float32)
    w3f[:, :D3_RAW] = np.asarray(w3, dtype=np.float32)
    b3f = np.zeros((D3,), dtype=np.float32)
    b3f[:D3_RAW] = b3

    np_dt = ml_dtypes.bfloat16 if mode == "bf16" else np.float32
    KS = _ks(mode)
    w1p = _pack_w(np.asarray(w1, dtype=np.float32), np_dt, FW1, KS_L1)
    w2p = _pack_w(np.asarray(w2, dtype=np.float32), np_dt, FW, KS)
    w3ps = {}
    fa = 0
    for gi, fgl in enumerate(L3_GROUPS):
        fw = fgl * P
        w3ps[f"w3g{gi}"] = _pack_w3_group(
            w3f[:, fa * P:fa * P + fw], np_dt, KS * FGL // fgl)
        fa += fgl
    b1p = np.ascontiguousarray(b1.reshape(D1 // P, P).T)
    b2p = np.ascontiguousarray(b2.reshape(D2 // P, P).T)
    b3p = np.ascontiguousarray(b3f.reshape(D3 // P, P).T)

    nc = build_nc(mode=mode)
    K0 = D0 // P
    in_maps = []
    for c in range(N_CORES):
        xs = x[c * B:(c + 1) * B]  # [B, D0]
        # xT[p, k, b] = x[b, k*128 + p]
        xT = np.ascontiguousarray(
            xs.reshape(B, K0, P).transpose(2, 1, 0)).astype(np_dt)
        in_maps.append({
            "xT": xT,
            "w1": w1p, "b1": b1p,
            "w2": w2p, "b2": b2p,
            "b3": b3p,
            **w3ps,
        })

    res = run_bass_kernel_spmd(
        nc, in_maps, core_ids=list(range(N_CORES)), trace=trace
    )
    LAST_RESULT = res
    outs = []
    for r in res.results:
        oT = r["out"]  # [P, F3, B]; logits[b, fg*128+p] = oT[p, fg, b]
        outs.append(oT.transpose(2, 1, 0).reshape(B, D3))
    out = np.concatenate(outs, axis=0)
    return np.ascontiguousarray(out[:, :D3_RAW].astype(np.float32))

